# revision 4
# baseline (speedup 1.0000x reference)
"""2-layer GCN (COO SpMM x2) on 8 Trainium2 NeuronCores.

Strategy (per core, dest-row sharding):
  - Nodes padded to 100352 = 8*98*128. Core c owns 12544 dest rows (98 blocks
    of 128).
  - Edges routed to the core owning their dest row. Per core, edges are
    grouped by (source bank, dest block); each (bank, block) cell is padded
    to a uniform number of 128-token groups (G_BB, computed from data) so
    all cores share one compiled module.
  - Per layer: dma_gather pulls 256B source rows (4 banks of 25024 rows keep
    indices in int16 range); the DVE expands compact (row-offset, val) pairs
    into val-weighted one-hot segment matrices S [128 tokens, 128 rows]; the
    PE computes psum += S^T @ G, accumulating all groups of a dest block;
    psum is added into an SBUF accumulator per block.
  - AllGather shares e1 across cores between layers; layer 2 repeats the
    same token schedule reading from the gathered e1.
  - Outputs per core: e1, e2, summed = x_shard + e1 + e2. e0 is the input.
"""
import os
import sys

sys.path.insert(0, "/opt/trn_rl_repo")

import numpy as np

N = 100001
NP = 100352          # padded nodes = 8 * 98 * 128
D = 64
CORES = 8
R_C = NP // CORES    # 12512 dest rows per core
NBLK = R_C // 128    # 98 dest blocks per core
BANKS = 4
BANK_R = NP // BANKS  # 25024 source rows per bank
BATCH = 4096         # tokens per dma_gather

LAST_EXEC_NS = None

_NC_CACHE = {}


def _build_module(G_BB):
    import concourse.bacc as bacc
    import concourse.mybir as mybir
    import concourse.tile as tile

    FP32, I16 = mybir.dt.float32, mybir.dt.int16

    G_BANK = NBLK * G_BB          # groups per bank
    T_BANK = G_BANK * 128         # tokens per bank
    G_TOT = BANKS * G_BANK        # groups per layer

    nc = bacc.Bacc("TRN2", target_bir_lowering=False, debug=False,
                   num_swdge_queues=4)
    x = nc.dram_tensor("x", [NP, D], FP32, kind="ExternalInput")
    idx = nc.dram_tensor("idx", [BANKS, 128, T_BANK // 16], I16,
                         kind="ExternalInput")
    roff = nc.dram_tensor("roff", [128, G_TOT], FP32, kind="ExternalInput")
    val = nc.dram_tensor("val", [128, G_TOT], FP32, kind="ExternalInput")
    iota = nc.dram_tensor("iota", [128, 128], FP32, kind="ExternalInput")
    e1_out = nc.dram_tensor("e1_out", [R_C, D], FP32, kind="ExternalOutput")
    e2_out = nc.dram_tensor("e2_out", [R_C, D], FP32, kind="ExternalOutput")
    sum_out = nc.dram_tensor("sum_out", [R_C, D], FP32, kind="ExternalOutput")

    x_shard = nc.dram_tensor("x_shard", [R_C, D], FP32, kind="ExternalInput")
    e1_bounce = nc.dram_tensor("e1_bounce", [R_C, D], FP32)
    e1_full = nc.dram_tensor("e1_full", [NP, D], FP32, addr_space="Shared")

    with tile.TileContext(nc) as tc:
        with tc.tile_pool(name="meta", bufs=1) as meta, \
             tc.tile_pool(name="ip", bufs=2) as ip, \
             tc.tile_pool(name="gp", bufs=3) as gp, \
             tc.tile_pool(name="sp", bufs=4) as sp, \
             tc.tile_pool(name="op", bufs=3) as op, \
             tc.tile_pool(name="pp", bufs=4, space="PSUM") as pp:

            iota_sb = meta.tile([128, 128], FP32)
            nc.sync.dma_start(out=iota_sb[:], in_=iota[:])
            roff_sb = meta.tile([128, G_TOT], FP32)
            nc.sync.dma_start(out=roff_sb[:], in_=roff[:])
            val_sb = meta.tile([128, G_TOT], FP32)
            nc.sync.dma_start(out=val_sb[:], in_=val[:])

            acc1 = meta.tile([128, NBLK, D], FP32)
            acc2 = meta.tile([128, NBLK, D], FP32)
            nc.vector.memset(acc1[:], 0.0)
            nc.vector.memset(acc2[:], 0.0)

            def layer(src_dram, acc):
                for bank in range(BANKS):
                    idx_sb = ip.tile([128, T_BANK // 16], I16, tag="idx")
                    nc.sync.dma_start(out=idx_sb[:], in_=idx[bank, :, :])
                    src_b = src_dram[bank * BANK_R:(bank + 1) * BANK_R, :]
                    nbatch = (T_BANK + BATCH - 1) // BATCH
                    psum_t = None
                    for nb in range(nbatch):
                        t0 = nb * BATCH
                        bsz = min(BATCH, T_BANK - t0)
                        g_t = gp.tile([128, bsz // 128, D], FP32, tag="g")
                        nc.gpsimd.dma_gather(
                            g_t[:], src_b,
                            idx_sb[:, t0 // 16:(t0 + bsz) // 16],
                            bsz, bsz, D, queue_num=nb % 4,
                            single_packet=False)
                        for k in range(bsz // 128):
                            g = bank * G_BANK + (t0 // 128) + k
                            gb = g - bank * G_BANK
                            blk = gb // G_BB
                            first = (gb % G_BB) == 0
                            last = (gb % G_BB) == G_BB - 1
                            s_t = sp.tile([128, 128], FP32, tag="s")
                            nc.vector.tensor_scalar(
                                out=s_t[:], in0=iota_sb[:],
                                scalar1=roff_sb[:, g:g + 1],
                                scalar2=val_sb[:, g:g + 1],
                                op0=mybir.AluOpType.is_equal,
                                op1=mybir.AluOpType.mult)
                            if first:
                                psum_t = pp.tile([128, D], FP32, tag="ps")
                            nc.tensor.matmul(psum_t[:], s_t[:], g_t[:, k, :],
                                             start=first, stop=last)
                            if last:
                                nc.vector.tensor_add(acc[:, blk, :],
                                                     acc[:, blk, :], psum_t[:])

            # Layer 1 from x
            layer(x, acc1)
            # publish e1: shard out + bounce for collective
            for blk in range(NBLK):
                nc.sync.dma_start(out=e1_out[blk * 128:(blk + 1) * 128, :],
                                  in_=acc1[:, blk, :])
                nc.sync.dma_start(out=e1_bounce[blk * 128:(blk + 1) * 128, :],
                                  in_=acc1[:, blk, :])
            with tc.tile_critical():
                cc_sem = nc.alloc_semaphore("cc_sem")
                nc.gpsimd.collective_compute(
                    "AllGather", mybir.AluOpType.bypass,
                    replica_groups=[list(range(CORES))],
                    ins=[e1_bounce.ap().opt()],
                    outs=[e1_full.ap().opt()],
                ).then_inc(cc_sem, 1)
                nc.gpsimd.wait_ge(cc_sem, 1)

            # Layer 2 from gathered e1
            layer(e1_full, acc2)

            # outputs: e2 and summed = x_shard + e1 + e2
            for blk in range(NBLK):
                nc.sync.dma_start(out=e2_out[blk * 128:(blk + 1) * 128, :],
                                  in_=acc2[:, blk, :])
                xs = op.tile([128, D], FP32, tag="xs")
                nc.sync.dma_start(out=xs[:], in_=x_shard[blk * 128:(blk + 1) * 128, :])
                st = op.tile([128, D], FP32, tag="st")
                nc.vector.tensor_add(st[:], acc1[:, blk, :], acc2[:, blk, :])
                nc.vector.tensor_add(st[:], st[:], xs[:])
                nc.sync.dma_start(out=sum_out[blk * 128:(blk + 1) * 128, :],
                                  in_=st[:])
    nc.compile()
    return nc


def kernel(row_idx, col_idx, adj_vals, emb_weight):
    global LAST_EXEC_NS
    from concourse.bass_utils import run_bass_kernel_spmd

    row = np.asarray(row_idx).astype(np.int64)
    col = np.asarray(col_idx).astype(np.int64)
    vals = np.asarray(adj_vals).astype(np.float32)
    emb = np.asarray(emb_weight).astype(np.float32)

    x_pad = np.zeros((NP, D), np.float32)
    x_pad[:N] = emb

    core = row // R_C
    bank = col // BANK_R
    blk = (row % R_C) >> 7
    roff_e = (row % R_C) & 127
    idx16 = (col - bank * BANK_R).astype(np.int16)

    cell = (core * BANKS + bank) * NBLK + blk       # global cell id
    ncell = CORES * BANKS * NBLK
    counts = np.bincount(cell, minlength=ncell)
    G_BB = int(np.ceil(counts.max() / 128))
    CAP = G_BB * 128

    order = np.argsort(cell, kind="stable")
    cell_sorted = cell[order]
    starts = np.zeros(ncell, np.int64)
    starts[1:] = np.cumsum(counts)[:-1]
    rank = np.arange(len(order)) - starts[cell_sorted]
    slot = cell_sorted * CAP + rank                  # unique slot per edge

    T_CORE = BANKS * NBLK * CAP
    G_TOT = BANKS * NBLK * G_BB
    idx_all = np.zeros(CORES * T_CORE, np.int16)
    roff_all = np.zeros(CORES * T_CORE, np.float32)
    val_all = np.zeros(CORES * T_CORE, np.float32)
    idx_all[slot] = idx16[order]
    roff_all[slot] = roff_e[order].astype(np.float32)
    val_all[slot] = vals[order]

    iota_np = np.tile(np.arange(128, dtype=np.float32), (128, 1)).copy()

    key = G_BB
    if key not in _NC_CACHE:
        _NC_CACHE[key] = _build_module(G_BB)
    nc = _NC_CACHE[key]

    in_maps = []
    for c in range(CORES):
        sl = slice(c * T_CORE, (c + 1) * T_CORE)
        idx_c = idx_all[sl]
        # per bank: [128, T_BANK//16] wrap-16 + replicate 8x
        T_BANK = NBLK * CAP
        idx_banks = np.stack([
            np.tile(idx_c[b * T_BANK:(b + 1) * T_BANK].reshape(-1, 16).T,
                    (8, 1))
            for b in range(BANKS)])
        roff_c = roff_all[sl].reshape(G_TOT, 128).T.copy()
        val_c = val_all[sl].reshape(G_TOT, 128).T.copy()
        in_maps.append({
            "x": x_pad,
            "x_shard": x_pad[c * R_C:(c + 1) * R_C],
            "idx": idx_banks,
            "roff": roff_c,
            "val": val_c,
            "iota": iota_np,
        })

    import time as _time
    nrep = int(os.environ.get("KBENCH_REPS", "1"))
    walls = []
    for _ in range(nrep):
        _t0 = _time.time()
        res = run_bass_kernel_spmd(nc, in_maps, core_ids=list(range(CORES)))
        walls.append(time_ns := int((_time.time() - _t0) * 1e9))
    globals()["RUN_WALLS"] = walls
    LAST_EXEC_NS = res.exec_time_ns

    e1 = np.concatenate([res.results[c]["e1_out"] for c in range(CORES)])[:N]
    e2 = np.concatenate([res.results[c]["e2_out"] for c in range(CORES)])[:N]
    summed = np.concatenate([res.results[c]["sum_out"] for c in range(CORES)])[:N]
    e0 = emb.copy()
    return (summed, e0, e1, e2)


# revision 7
# speedup vs baseline: 2197.7904x; 2197.7904x over previous
"""2-layer GCN (COO SpMM x2) on 8 Trainium2 NeuronCores.

Strategy (per core, dest-row sharding):
  - Nodes padded to 100352 = 8*98*128. Core c owns 12544 dest rows (98 blocks
    of 128).
  - Edges routed to the core owning their dest row. Per core, edges are
    grouped by (source bank, dest block); each (bank, block) cell is padded
    to a uniform number of 128-token groups (G_BB, computed from data) so
    all cores share one compiled module.
  - Per layer: dma_gather pulls 256B source rows (4 banks of 25024 rows keep
    indices in int16 range); the DVE expands compact (row-offset, val) pairs
    into val-weighted one-hot segment matrices S [128 tokens, 128 rows]; the
    PE computes psum += S^T @ G, accumulating all groups of a dest block;
    psum is added into an SBUF accumulator per block.
  - AllGather shares e1 across cores between layers; layer 2 repeats the
    same token schedule reading from the gathered e1.
  - Outputs per core: e1, e2, summed = x_shard + e1 + e2. e0 is the input.
"""
import os
import sys

sys.path.insert(0, "/opt/trn_rl_repo")

import numpy as np

N = 100001
NP = 100352          # padded nodes = 8 * 98 * 128
D = 64
CORES = 8
R_C = NP // CORES    # 12512 dest rows per core
NBLK = R_C // 128    # 98 dest blocks per core
BANKS = 4
BANK_R = NP // BANKS  # 25024 source rows per bank
BATCH = 4096         # tokens per dma_gather

LAST_EXEC_NS = None

_NC_CACHE = {}


def _build_module(G_BB):
    import concourse.bacc as bacc
    import concourse.mybir as mybir
    import concourse.tile as tile

    FP32, I16 = mybir.dt.float32, mybir.dt.int16

    G_BANK = NBLK * G_BB          # groups per bank
    T_BANK = G_BANK * 128         # tokens per bank
    G_TOT = BANKS * G_BANK        # groups per layer

    nc = bacc.Bacc("TRN2", target_bir_lowering=False, debug=False,
                   num_swdge_queues=4)
    x = nc.dram_tensor("x", [NP, D], FP32, kind="ExternalInput")
    idx = nc.dram_tensor("idx", [BANKS, 128, T_BANK // 16], I16,
                         kind="ExternalInput")
    roff = nc.dram_tensor("roff", [128, G_TOT], FP32, kind="ExternalInput")
    val = nc.dram_tensor("val", [128, G_TOT], FP32, kind="ExternalInput")
    iota = nc.dram_tensor("iota", [128, 128], FP32, kind="ExternalInput")
    e1_out = nc.dram_tensor("e1_out", [R_C, D], FP32, kind="ExternalOutput")
    e2_out = nc.dram_tensor("e2_out", [R_C, D], FP32, kind="ExternalOutput")
    sum_out = nc.dram_tensor("sum_out", [R_C, D], FP32, kind="ExternalOutput")

    x_shard = nc.dram_tensor("x_shard", [R_C, D], FP32, kind="ExternalInput")
    e1_bounce = nc.dram_tensor("e1_bounce", [R_C, D], FP32)
    e1_full = nc.dram_tensor("e1_full", [NP, D], FP32, addr_space="Shared")

    with tile.TileContext(nc) as tc:
        with tc.tile_pool(name="meta", bufs=1) as meta, \
             tc.tile_pool(name="ip", bufs=2) as ip, \
             tc.tile_pool(name="gp", bufs=3) as gp, \
             tc.tile_pool(name="sp", bufs=4) as sp, \
             tc.tile_pool(name="op", bufs=3) as op, \
             tc.tile_pool(name="pp", bufs=4, space="PSUM") as pp:

            iota_sb = meta.tile([128, 128], FP32)
            nc.sync.dma_start(out=iota_sb[:], in_=iota[:])
            roff_sb = meta.tile([128, G_TOT], FP32)
            nc.sync.dma_start(out=roff_sb[:], in_=roff[:])
            val_sb = meta.tile([128, G_TOT], FP32)
            nc.sync.dma_start(out=val_sb[:], in_=val[:])

            acc1 = meta.tile([128, NBLK, D], FP32)
            acc2 = meta.tile([128, NBLK, D], FP32)
            nc.vector.memset(acc1[:], 0.0)
            nc.vector.memset(acc2[:], 0.0)

            def layer(src_dram, acc):
                for bank in range(BANKS):
                    idx_sb = ip.tile([128, T_BANK // 16], I16, tag="idx")
                    nc.sync.dma_start(out=idx_sb[:], in_=idx[bank, :, :])
                    src_b = src_dram[bank * BANK_R:(bank + 1) * BANK_R, :]
                    nbatch = (T_BANK + BATCH - 1) // BATCH
                    psum_t = None
                    for nb in range(nbatch):
                        t0 = nb * BATCH
                        bsz = min(BATCH, T_BANK - t0)
                        g_t = gp.tile([128, bsz // 128, D], FP32, tag="g")
                        nc.gpsimd.dma_gather(
                            g_t[:], src_b,
                            idx_sb[:, t0 // 16:(t0 + bsz) // 16],
                            bsz, bsz, D, queue_num=nb % 4,
                            single_packet=False)
                        ngr = bsz // 128
                        SB = 8  # groups per S-build op
                        s_tiles = []
                        for j0 in range(0, ngr, SB):
                            jn = min(SB, ngr - j0)
                            g0 = bank * G_BANK + (t0 // 128) + j0
                            s_t = sp.tile([128, SB, 128], FP32, tag="s")
                            io3 = iota_sb[:, None, :].broadcast_to([128, jn, 128])
                            ro3 = roff_sb[:, g0:g0 + jn, None].broadcast_to(
                                [128, jn, 128])
                            va3 = val_sb[:, g0:g0 + jn, None].broadcast_to(
                                [128, jn, 128])
                            nc.vector.tensor_tensor(
                                out=s_t[:, :jn, :], in0=io3, in1=ro3,
                                op=mybir.AluOpType.is_equal)
                            # val multiply on ACT (idle engine; per-partition
                            # scale) to keep DVE off the POOL-shared SBUF port
                            for jj in range(jn):
                                nc.scalar.activation(
                                    out=s_t[:, jj, :], in_=s_t[:, jj, :],
                                    func=mybir.ActivationFunctionType.Copy,
                                    scale=val_sb[:, g0 + jj:g0 + jj + 1])
                            s_tiles.append(s_t)
                        for k in range(ngr):
                            g = bank * G_BANK + (t0 // 128) + k
                            gb = g - bank * G_BANK
                            blk = gb // G_BB
                            first = (gb % G_BB) == 0
                            last = (gb % G_BB) == G_BB - 1
                            if first:
                                psum_t = pp.tile([128, D], FP32, tag="ps")
                            nc.tensor.matmul(psum_t[:],
                                             s_tiles[k // SB][:, k % SB, :],
                                             g_t[:, k, :],
                                             start=first, stop=last)
                            if last:
                                nc.vector.tensor_add(acc[:, blk, :],
                                                     acc[:, blk, :], psum_t[:])

            skip_ag = os.environ.get("KSKIP_AG") == "1"
            # Layer 1 from x
            layer(x, acc1)
            # publish e1: shard out + bounce for collective
            for blk in range(NBLK):
                nc.sync.dma_start(out=e1_out[blk * 128:(blk + 1) * 128, :],
                                  in_=acc1[:, blk, :])
                nc.sync.dma_start(out=e1_bounce[blk * 128:(blk + 1) * 128, :],
                                  in_=acc1[:, blk, :])
            if not skip_ag:
                with tc.tile_critical():
                    cc_sem = nc.alloc_semaphore("cc_sem")
                    nc.gpsimd.collective_compute(
                        "AllGather", mybir.AluOpType.bypass,
                        replica_groups=[list(range(CORES))],
                        ins=[e1_bounce.ap().opt()],
                        outs=[e1_full.ap().opt()],
                    ).then_inc(cc_sem, 1)
                    nc.gpsimd.wait_ge(cc_sem, 1)
            else:
                nc.sync.dma_start(out=e1_full[:R_C, :], in_=e1_bounce[:])

            # Layer 2 from gathered e1
            layer(e1_full, acc2)

            # outputs: e2 and summed = x_shard + e1 + e2
            for blk in range(NBLK):
                nc.sync.dma_start(out=e2_out[blk * 128:(blk + 1) * 128, :],
                                  in_=acc2[:, blk, :])
                xs = op.tile([128, D], FP32, tag="xs")
                nc.sync.dma_start(out=xs[:], in_=x_shard[blk * 128:(blk + 1) * 128, :])
                st = op.tile([128, D], FP32, tag="st")
                nc.vector.tensor_add(st[:], acc1[:, blk, :], acc2[:, blk, :])
                nc.vector.tensor_add(st[:], st[:], xs[:])
                nc.sync.dma_start(out=sum_out[blk * 128:(blk + 1) * 128, :],
                                  in_=st[:])
    nc.compile()
    return nc


def kernel(row_idx, col_idx, adj_vals, emb_weight):
    global LAST_EXEC_NS
    from concourse.bass_utils import run_bass_kernel_spmd

    row = np.asarray(row_idx).astype(np.int64)
    col = np.asarray(col_idx).astype(np.int64)
    vals = np.asarray(adj_vals).astype(np.float32)
    emb = np.asarray(emb_weight).astype(np.float32)

    x_pad = np.zeros((NP, D), np.float32)
    x_pad[:N] = emb

    core = row // R_C
    bank = col // BANK_R
    blk = (row % R_C) >> 7
    roff_e = (row % R_C) & 127
    idx16 = (col - bank * BANK_R).astype(np.int16)

    cell = (core * BANKS + bank) * NBLK + blk       # global cell id
    ncell = CORES * BANKS * NBLK
    counts = np.bincount(cell, minlength=ncell)
    G_BB = int(np.ceil(counts.max() / 128))
    CAP = G_BB * 128

    order = np.argsort(cell, kind="stable")
    cell_sorted = cell[order]
    starts = np.zeros(ncell, np.int64)
    starts[1:] = np.cumsum(counts)[:-1]
    rank = np.arange(len(order)) - starts[cell_sorted]
    slot = cell_sorted * CAP + rank                  # unique slot per edge

    T_CORE = BANKS * NBLK * CAP
    G_TOT = BANKS * NBLK * G_BB
    idx_all = np.zeros(CORES * T_CORE, np.int16)
    roff_all = np.zeros(CORES * T_CORE, np.float32)
    val_all = np.zeros(CORES * T_CORE, np.float32)
    idx_all[slot] = idx16[order]
    roff_all[slot] = roff_e[order].astype(np.float32)
    val_all[slot] = vals[order]

    iota_np = np.tile(np.arange(128, dtype=np.float32), (128, 1)).copy()

    key = (G_BB, os.environ.get("KSKIP_AG") == "1")
    if key not in _NC_CACHE:
        _NC_CACHE[key] = _build_module(G_BB)
    nc = _NC_CACHE[key]

    in_maps = []
    for c in range(CORES):
        sl = slice(c * T_CORE, (c + 1) * T_CORE)
        idx_c = idx_all[sl]
        # per bank: [128, T_BANK//16] wrap-16 + replicate 8x
        T_BANK = NBLK * CAP
        idx_banks = np.stack([
            np.tile(idx_c[b * T_BANK:(b + 1) * T_BANK].reshape(-1, 16).T,
                    (8, 1))
            for b in range(BANKS)])
        roff_c = roff_all[sl].reshape(G_TOT, 128).T.copy()
        val_c = val_all[sl].reshape(G_TOT, 128).T.copy()
        in_maps.append({
            "x": x_pad,
            "x_shard": x_pad[c * R_C:(c + 1) * R_C],
            "idx": idx_banks,
            "roff": roff_c,
            "val": val_c,
            "iota": iota_np,
        })

    import time as _time
    nrep = int(os.environ.get("KBENCH_REPS", "1"))
    walls = []
    for _ in range(nrep):
        _t0 = _time.time()
        res = run_bass_kernel_spmd(nc, in_maps, core_ids=list(range(CORES)))
        walls.append(time_ns := int((_time.time() - _t0) * 1e9))
    globals()["RUN_WALLS"] = walls
    LAST_EXEC_NS = res.exec_time_ns

    e1 = np.concatenate([res.results[c]["e1_out"] for c in range(CORES)])[:N]
    e2 = np.concatenate([res.results[c]["e2_out"] for c in range(CORES)])[:N]
    summed = np.concatenate([res.results[c]["sum_out"] for c in range(CORES)])[:N]
    e0 = emb.copy()
    return (summed, e0, e1, e2)
